# revision 28
# baseline (speedup 1.0000x reference)
"""MoE layer (8 experts, top-2) on 8 Trainium2 NeuronCores, expert-parallel.

Strategy
--------
Host (dispatch): compute router logits/top-k on host, gather each expert's
tokens into a padded capacity buffer (C = max expert load, 64-aligned),
pre-pack activations/weights into the exact SBUF tile layout
(partition-major) in fp16.
Device (one expert per core, SPMD): Y_e = w_down[e] @ (silu(w_gate[e] @ x_e)
* (w_up[e] @ x_e)) over the expert's C gathered tokens; all matmuls fp16
inputs with fp32 PSUM accumulation (fp16 runs at full PE rate like bf16 but
with 8x finer mantissa). Token columns are processed in 512-wide blocks, the
remainder merged into the last block so both share one pass over the
restreamed weights.
Host (combine): scatter-add per-token routing-weighted outputs.
"""

import os
import numpy as np
from contextlib import ExitStack

H = 2048
I = 5632
E = 8
P = 128
NB = 512  # token block (matmul free dim / PSUM bank)

KH = H // P   # 16  k-tiles over H
MI = I // P   # 44  m-tiles over I

DT = np.float16  # fp16: PE full rate like bf16, 8x finer mantissa


def _superblocks(C):
    """Column groups; a trailing remainder (<NB) is merged into the last
    full block so both share one pass over the weights."""
    blocks = []
    t = 0
    while t < C:
        blocks.append((t, min(NB, C - t)))
        t += NB
    sbs = [[b] for b in blocks]
    if len(sbs) >= 2 and sbs[-1][0][1] < NB:
        tail = sbs.pop()[0]
        sbs[-1].append(tail)
    return sbs


def build_program(C, h=H, i_dim=I, sim_safe_act=False):
    """Build the SPMD bass program for one expert over C tokens.

    DRAM I/O layouts (all partition-major, pre-packed on host):
      x  [P, KH, C]        fp16   x[p, k, t]  = token t, hidden 128k+p
      wg [MI, P, KH*P]     fp16   wg[m, p, kf] (kf = k*128+f): w_gate.T tiles
      wu [MI, P, KH*P]     fp16   same for w_up
      wd [KH, P, MI*P]     fp16   w_down.T tiles
      y  [P, KH, C]        f32    y[p, m2, t] = output hidden 128*m2+p
    """
    from concourse import bacc, tile, mybir

    kh = h // P
    mi = i_dim // P
    bf = mybir.dt.float16
    f32 = mybir.dt.float32
    Silu = mybir.ActivationFunctionType.Silu

    nc = bacc.Bacc(None)
    X = nc.declare_dram_parameter("x", [P, kh, C], bf, isOutput=False)
    WG = nc.declare_dram_parameter("wg", [mi, P, kh * P], bf, isOutput=False)
    WU = nc.declare_dram_parameter("wu", [mi, P, kh * P], bf, isOutput=False)
    WD = nc.declare_dram_parameter("wd", [kh, P, mi * P], bf, isOutput=False)
    Y = nc.declare_dram_parameter("y", [P, kh, C], f32, isOutput=True)

    with ExitStack() as ctx:
        tc = ctx.enter_context(tile.TileContext(nc))
        xpool = ctx.enter_context(tc.tile_pool(name="xpool", bufs=2))
        wpool = ctx.enter_context(tc.tile_pool(name="wpool", bufs=6))
        dpool = ctx.enter_context(tc.tile_pool(name="dpool", bufs=4))
        hpool = ctx.enter_context(tc.tile_pool(name="hpool", bufs=1))
        apool = ctx.enter_context(tc.tile_pool(name="apool", bufs=3))
        ypool = ctx.enter_context(tc.tile_pool(name="ypool", bufs=3))
        pg_pool = ctx.enter_context(tc.tile_pool(name="pg", bufs=3, space="PSUM"))
        pu_pool = ctx.enter_context(tc.tile_pool(name="pu", bufs=3, space="PSUM"))
        py_pool = ctx.enter_context(tc.tile_pool(name="py", bufs=2, space="PSUM"))
        wupool = ctx.enter_context(tc.tile_pool(name="wupool", bufs=1))

        # warmup: dummy matmul chain on garbage SBUF ramps the PE clock
        # p-state while the first weight/x DMAs are still in flight
        wu_w = wupool.tile([P, P], bf, tag="wu_w", name="wu_w")
        wu_x = wupool.tile([P, NB], bf, tag="wu_x", name="wu_x")
        nc.vector.memset(wu_w[:, :], 0.0)
        nc.vector.memset(wu_x[:, :], 0.0)
        wu_p = pg_pool.tile([P, NB], f32, tag="pg")
        for k in range(kh):
            nc.tensor.matmul(wu_p[:, :], wu_w[:, :], wu_x[:, :],
                             start=(k == 0), stop=(k == kh - 1))

        first_sb = True
        wg0 = wu0 = None
        for sb in _superblocks(C):
            q = kh * P // 4
            if first_sb:
                # prefetch m=0 weights ahead of the x stream so the PE's
                # first chain starts ~1us after launch, not behind 2MB of x
                wg0 = wpool.tile([P, kh * P], bf, tag="wg_t")
                for j in range(4):
                    nc.sync.dma_start(wg0[:, j * q : (j + 1) * q], WG[0, :, j * q : (j + 1) * q])
                wu0 = wpool.tile([P, kh * P], bf, tag="wu_t")
                for j in range(4):
                    nc.gpsimd.dma_start(wu0[:, j * q : (j + 1) * q], WU[0, :, j * q : (j + 1) * q])
            # ---- load X for each column group: kh tiles [P, tn]
            x_ts = []
            for g, (t0, tn) in enumerate(sb):
                x_t = xpool.tile([P, kh, tn], bf, tag=f"x_t{g}", name=f"x_t{g}")
                for k in range(kh):
                    eng = nc.scalar if k % 2 == 0 else nc.gpsimd
                    eng.dma_start(x_t[:, k, :tn], X[:, k, t0 : t0 + tn])
                x_ts.append(x_t)

            # ---- mm1/mm2 + silu*mul -> h (one weight pass for all groups)
            h_ts = [
                hpool.tile([P, mi, sb[g][1]], bf, tag=f"h{g}", name=f"h_t{g}")
                for g in range(len(sb))
            ]
            for m in range(mi):
                if first_sb and m == 0:
                    wg_t, wu_t = wg0, wu0
                else:
                    wg_t = wpool.tile([P, kh * P], bf, tag="wg_t")
                    for j in range(4):
                        nc.sync.dma_start(wg_t[:, j * q : (j + 1) * q], WG[m, :, j * q : (j + 1) * q])
                    wu_t = wpool.tile([P, kh * P], bf, tag="wu_t")
                    for j in range(4):
                        nc.gpsimd.dma_start(wu_t[:, j * q : (j + 1) * q], WU[m, :, j * q : (j + 1) * q])

                pgs, pus = [], []
                for g, (t0, tn) in enumerate(sb):
                    pg = pg_pool.tile([P, NB], f32, tag="pg")
                    pgs.append(pg)
                    for k in range(kh):
                        nc.tensor.matmul(
                            pg[:, :tn],
                            wg_t[:, k * P : (k + 1) * P],
                            x_ts[g][:, k, :tn],
                            start=(k == 0),
                            stop=(k == kh - 1),
                        )
                for g, (t0, tn) in enumerate(sb):
                    pu = pu_pool.tile([P, NB], f32, tag="pu")
                    pus.append(pu)
                    for k in range(kh):
                        nc.tensor.matmul(
                            pu[:, :tn],
                            wu_t[:, k * P : (k + 1) * P],
                            x_ts[g][:, k, :tn],
                            start=(k == 0),
                            stop=(k == kh - 1),
                        )
                for g, (t0, tn) in enumerate(sb):
                    pg, pu = pgs[g], pus[g]
                    g_act = apool.tile([P, NB], f32, tag="g_act")
                    if sim_safe_act:
                        # silu(g) = g * sigmoid(g); CoreSim lacks the Silu LUT
                        nc.scalar.activation(
                            g_act[:, :tn],
                            pg[:, :tn],
                            mybir.ActivationFunctionType.Sigmoid,
                        )
                        nc.vector.tensor_mul(g_act[:, :tn], g_act[:, :tn], pg[:, :tn])
                    else:
                        nc.scalar.activation(g_act[:, :tn], pg[:, :tn], Silu)
                    nc.vector.tensor_mul(h_ts[g][:, m, :tn], g_act[:, :tn], pu[:, :tn])

            # ---- mm3 -> y (one weight pass for all groups)
            for m2 in range(kh):
                dhalf = mi * P // 2
                wd_t = dpool.tile([P, mi * P], bf, tag="wd_t")
                nc.sync.dma_start(wd_t[:, :dhalf], WD[m2, :, :dhalf])
                nc.gpsimd.dma_start(wd_t[:, dhalf:], WD[m2, :, dhalf:])
                # tail group first: its py chains are slot-constrained, so
                # bury them behind the full-rate main-group stream
                for g, (t0, tn) in reversed(list(enumerate(sb))):
                    py = py_pool.tile([P, NB], f32, tag="py")
                    for k2 in range(mi):
                        nc.tensor.matmul(
                            py[:, :tn],
                            wd_t[:, k2 * P : (k2 + 1) * P],
                            h_ts[g][:, k2, :tn],
                            start=(k2 == 0),
                            stop=(k2 == mi - 1),
                        )
                    y_sb = ypool.tile([P, NB], f32, tag="y_sb")
                    nc.vector.tensor_copy(y_sb[:, :tn], py[:, :tn])
                    nc.scalar.dma_start(Y[:, m2, t0 : t0 + tn], y_sb[:, :tn])
            first_sb = False

    nc.compile()
    return nc


MI2 = MI // 2  # 22 m-tiles over I/2
KH2 = KH // 2  # 8 k-tiles over H/2
BLK = 960      # strassen column block (halves of 480 fit one PSUM bank; a
               # C=2104 tail of 184 has 92-wide halves, wide enough that
               # LDWEIGHTS still hides under the tail matmuls)


def build_program_strassen(C):
    """Strassen(1-level) mm1/mm2 + dense mm3, h staged through DRAM.

    Columns: two BLK blocks + tail (C - 2*BLK, halves tail//2), tail merged
    into pass 1 so it shares that pass's A-weight stream.

    DRAM I/O (fp16 unless noted):
      x  [P, KH, C]            x[p,k,t] = token t, hidden 128k+p
      ag [7, MI2, P, KH2*P]    Strassen A-combos of w_gate (lhsT tiles)
      au [7, MI2, P, KH2*P]    same for w_up
      wd [KH, P, MI*P]         w_down.T tiles (dense mm3)
      y  [P, KH, C]  f32       output
      hd [P, MI, C]            internal scratch: h = silu(g)*u
    """
    from concourse import bacc, tile, mybir

    bf = mybir.dt.float16
    f32 = mybir.dt.float32
    Silu = mybir.ActivationFunctionType.Silu

    assert C >= 2 * BLK
    tail = C - 2 * BLK
    th = tail // 2
    hn = BLK // 2

    nc = bacc.Bacc(None)
    X = nc.declare_dram_parameter("x", [P, KH, C], bf, isOutput=False)
    AG = nc.declare_dram_parameter("ag", [7, MI2, P, KH2 * P], bf, isOutput=False)
    AU = nc.declare_dram_parameter("au", [7, MI2, P, KH2 * P], bf, isOutput=False)
    WD = nc.declare_dram_parameter("wd", [KH, P, MI * P], bf, isOutput=False)
    Y = nc.declare_dram_parameter("y", [P, KH, C], f32, isOutput=True)
    HD = nc.dram_tensor("hd", [P, MI, C], bf)

    with ExitStack() as ctx:
        tc = ctx.enter_context(tile.TileContext(nc))
        xpool = ctx.enter_context(tc.tile_pool(name="xpool", bufs=1))
        xtpool = ctx.enter_context(tc.tile_pool(name="xtpool", bufs=1))
        spool = ctx.enter_context(tc.tile_pool(name="spool", bufs=1))
        stpool = ctx.enter_context(tc.tile_pool(name="stpool", bufs=1))
        apool = ctx.enter_context(tc.tile_pool(name="apool", bufs=8))
        gupool = ctx.enter_context(tc.tile_pool(name="gupool", bufs=2))
        gutpool = ctx.enter_context(tc.tile_pool(name="gutpool", bufs=2))
        hopool = ctx.enter_context(tc.tile_pool(name="hopool", bufs=2))
        hotpool = ctx.enter_context(tc.tile_pool(name="hotpool", bufs=2))
        hipool = ctx.enter_context(tc.tile_pool(name="hipool", bufs=3))
        htpool = ctx.enter_context(tc.tile_pool(name="htpool", bufs=1))
        dpool = ctx.enter_context(tc.tile_pool(name="dpool", bufs=2))
        ypool = ctx.enter_context(tc.tile_pool(name="ypool", bufs=2))
        wupool = ctx.enter_context(tc.tile_pool(name="wupool", bufs=1))
        ppool = ctx.enter_context(tc.tile_pool(name="pp", bufs=8, space="PSUM"))

        # warmup: dummy chain ramps PE clock while first DMAs fly
        wu_w = wupool.tile([P, P], bf, tag="wu_w", name="wu_w")
        wu_x = wupool.tile([P, NB], bf, tag="wu_x", name="wu_x")
        nc.vector.memset(wu_w[:, :], 0.0)
        nc.vector.memset(wu_x[:, :], 0.0)
        for _ in range(4):
            wu_p = ppool.tile([P, NB], f32, tag="pp")
            for k in range(KH):
                nc.tensor.matmul(wu_p[:, :], wu_w[:, :], wu_x[:, :],
                                 start=(k == 0), stop=(k == KH - 1))

        def emit_combines(eng, P_, oA, oB, w):
            """G-quadrant combines; DVE may read only ONE PSUM operand per op,
            so accumulate into SBUF. Op order frees PSUM banks ahead of the
            next product round's ring slots (P1 first, then P2, P7, P4, ...).
            oA = row-block m [P, 2w] (C11|C12), oB = row-block m+MI2 (C21|C22)."""
            A_lo, A_hi = oA[:, 0:w], oA[:, w:2 * w]
            B_lo, B_hi = oB[:, 0:w], oB[:, w:2 * w]
            p = [t[:, 0:w] for t in P_]
            eng.tensor_copy(A_lo, p[0])            # C11 := P1
            eng.tensor_copy(B_hi, p[0])            # C22 := P1        [P1 free]
            eng.tensor_sub(B_hi, B_hi, p[1])       # C22 -= P2
            eng.tensor_copy(B_lo, p[1])            # C21 := P2        [P2 free]
            eng.tensor_add(A_lo, A_lo, p[6])       # C11 += P7        [P7 free]
            eng.tensor_add(B_lo, B_lo, p[3])       # C21 += P4
            eng.tensor_add(A_lo, A_lo, p[3])       # C11 += P4        [P4 free]
            eng.tensor_add(B_hi, B_hi, p[2])       # C22 += P3
            eng.tensor_copy(A_hi, p[2])            # C12 := P3        [P3 free]
            eng.tensor_add(A_hi, A_hi, p[4])       # C12 += P5
            eng.tensor_sub(A_lo, A_lo, p[4])       # C11 -= P5        [P5 free]
            eng.tensor_add(B_hi, B_hi, p[5])       # C22 += P6        [P6 free]

        for pidx in range(2):
            t0 = pidx * BLK
            has_tail = (pidx == 1) and tail > 0

            # ---- x for this block (+ tail)
            x_t = xpool.tile([P, KH, BLK], bf, tag="x", name=f"x_b{pidx}")
            for k in range(KH):
                eng = nc.scalar if k % 2 == 0 else nc.gpsimd
                eng.dma_start(x_t[:, k, :], X[:, k, t0:t0 + BLK])
            if has_tail:
                xt = xtpool.tile([P, KH, tail], bf, tag="xt", name="x_tail")
                nc.gpsimd.dma_start(xt[:, :, :], X[:, :, 2 * BLK:2 * BLK + tail])

            # ---- S combos (vector); S2/S5 are raw views of x
            def build_s(xsrc, w, pool, tg):
                s1 = pool.tile([P, KH2, w], bf, tag=f"s1{tg}")
                nc.vector.tensor_add(s1[:, :, :], xsrc[:, 0:KH2, 0:w], xsrc[:, KH2:KH, w:2 * w])
                s3 = pool.tile([P, KH2, w], bf, tag=f"s3{tg}")
                nc.vector.tensor_sub(s3[:, :, :], xsrc[:, 0:KH2, w:2 * w], xsrc[:, KH2:KH, w:2 * w])
                s4 = pool.tile([P, KH2, w], bf, tag=f"s4{tg}")
                nc.vector.tensor_sub(s4[:, :, :], xsrc[:, KH2:KH, 0:w], xsrc[:, 0:KH2, 0:w])
                s6 = pool.tile([P, KH2, w], bf, tag=f"s6{tg}")
                nc.vector.tensor_add(s6[:, :, :], xsrc[:, 0:KH2, 0:w], xsrc[:, 0:KH2, w:2 * w])
                s7 = pool.tile([P, KH2, w], bf, tag=f"s7{tg}")
                nc.vector.tensor_add(s7[:, :, :], xsrc[:, KH2:KH, 0:w], xsrc[:, KH2:KH, w:2 * w])
                # mov(i, k) -> AP for product i, k-tile k
                def mov(i, k):
                    return [
                        lambda: s1[:, k, :],
                        lambda: xsrc[:, k, 0:w],            # S2 = X11
                        lambda: s3[:, k, :],
                        lambda: s4[:, k, :],
                        lambda: xsrc[:, KH2 + k, w:2 * w],  # S5 = X22
                        lambda: s6[:, k, :],
                        lambda: s7[:, k, :],
                    ][i]()
                return mov

            mov = build_s(x_t, hn, spool, "")
            if has_tail:
                movt = build_s(xt, th, stpool, "t")

            # ---- mm1/mm2 strassen m-loop
            for m in range(MI2):
                for W_, oAB, oABt in ((AG, "g", "gt"), (AU, "u", "ut")):
                    a_ts = []
                    for i in range(7):
                        a_t = apool.tile([P, KH2 * P], bf, tag="a")
                        half = KH2 * P // 2
                        nc.sync.dma_start(a_t[:, :half], W_[i, m, :, :half])
                        nc.gpsimd.dma_start(a_t[:, half:], W_[i, m, :, half:])
                        a_ts.append(a_t)
                    # block products
                    P_ = []
                    for i in range(7):
                        pp = ppool.tile([P, NB], f32, tag="pp")
                        for k in range(KH2):
                            nc.tensor.matmul(
                                pp[:, :hn], a_ts[i][:, k * P:(k + 1) * P], mov(i, k),
                                start=(k == 0), stop=(k == KH2 - 1))
                        P_.append(pp)
                    oA = gupool.tile([P, BLK], bf, tag=f"{oAB}A")
                    oB = gupool.tile([P, BLK], bf, tag=f"{oAB}B")
                    emit_combines(nc.vector, P_, oA, oB, hn)
                    if oAB == "g":
                        gA, gB = oA, oB
                    else:
                        uA, uB = oA, oB
                    if has_tail:
                        Pt = []
                        for i in range(7):
                            pp = ppool.tile([P, NB], f32, tag="pp")
                            for k in range(KH2):
                                nc.tensor.matmul(
                                    pp[:, :th], a_ts[i][:, k * P:(k + 1) * P], movt(i, k),
                                    start=(k == 0), stop=(k == KH2 - 1))
                            Pt.append(pp)
                        oAt = gutpool.tile([P, tail], bf, tag=f"{oABt}A")
                        oBt = gutpool.tile([P, tail], bf, tag=f"{oABt}B")
                        emit_combines(nc.vector, Pt, oAt, oBt, th)
                        if oAB == "g":
                            gAt, gBt = oAt, oBt
                        else:
                            uAt, uBt = oAt, oBt

                # silu in-place on g, then h = g*u -> DRAM
                nc.scalar.activation(gA[:, :], gA[:, :], Silu)
                nc.scalar.activation(gB[:, :], gB[:, :], Silu)
                hA = hopool.tile([P, BLK], bf, tag="hA")
                nc.gpsimd.tensor_mul(hA[:, :], gA[:, :], uA[:, :])
                nc.scalar.dma_start(HD[:, m, t0:t0 + BLK], hA[:, :])
                hB = hopool.tile([P, BLK], bf, tag="hB")
                nc.gpsimd.tensor_mul(hB[:, :], gB[:, :], uB[:, :])
                nc.scalar.dma_start(HD[:, m + MI2, t0:t0 + BLK], hB[:, :])
                if has_tail:
                    nc.scalar.activation(gAt[:, :], gAt[:, :], Silu)
                    nc.scalar.activation(gBt[:, :], gBt[:, :], Silu)
                    hAt = hotpool.tile([P, tail], bf, tag="hAt")
                    nc.vector.tensor_mul(hAt[:, :], gAt[:, :], uAt[:, :])
                    nc.scalar.dma_start(HD[:, m, 2 * BLK:2 * BLK + tail], hAt[:, :])
                    hBt = hotpool.tile([P, tail], bf, tag="hBt")
                    nc.vector.tensor_mul(hBt[:, :], gBt[:, :], uBt[:, :])
                    nc.scalar.dma_start(HD[:, m + MI2, 2 * BLK:2 * BLK + tail], hBt[:, :])

            # ---- mm3 (dense) for this block (+ tail in pass 1)
            chunks = [(0, 160), (160, 160), (320, 160), (480, 160), (640, 160), (800, 160)]
            groups = [chunks[0:3], chunks[3:6]]
            third = (MI // 3) + 1  # 15
            for gi, group in enumerate(groups):
                h_ins = []
                for (co, cn) in group:
                    hq = hipool.tile([P, MI, cn], bf, tag="hq")
                    # split the chunk load across all three DMA queues
                    nc.gpsimd.dma_start(hq[:, :third, :], HD[:, :third, t0 + co:t0 + co + cn])
                    nc.sync.dma_start(hq[:, third:2 * third, :], HD[:, third:2 * third, t0 + co:t0 + co + cn])
                    nc.scalar.dma_start(hq[:, 2 * third:, :], HD[:, 2 * third:, t0 + co:t0 + co + cn])
                    h_ins.append(hq)
                tail_here = has_tail and gi == len(groups) - 1
                if tail_here:
                    htq = htpool.tile([P, MI, tail], bf, tag="htq")
                    nc.gpsimd.dma_start(htq[:, :, :], HD[:, :, 2 * BLK:2 * BLK + tail])
                for m2 in range(KH):
                    dhalf = MI * P // 2
                    wd_t = dpool.tile([P, MI * P], bf, tag="wd_t")
                    nc.sync.dma_start(wd_t[:, :dhalf], WD[m2, :, :dhalf])
                    nc.gpsimd.dma_start(wd_t[:, dhalf:], WD[m2, :, dhalf:])
                    todo = [(co, cn, hq) for (co, cn), hq in zip(group, h_ins)]
                    if tail_here:
                        todo.append((2 * BLK - t0, tail, htq))
                    for (co, cn, hq) in todo:
                        py = ppool.tile([P, NB], f32, tag="pp")
                        for k2 in range(MI):
                            nc.tensor.matmul(
                                py[:, :cn], wd_t[:, k2 * P:(k2 + 1) * P], hq[:, k2, :cn],
                                start=(k2 == 0), stop=(k2 == MI - 1))
                        y_sb = ypool.tile([P, max(160, tail)], f32, tag="y_sb")
                        nc.vector.tensor_copy(y_sb[:, :cn], py[:, :cn])
                        nc.scalar.dma_start(Y[:, m2, t0 + co:t0 + co + cn], y_sb[:, :cn])

    nc.compile()
    return nc


def _pack_strassen_w1(w):
    """[I, H] -> [7, MI2, P, KH2*P] strassen A-combos, lhsT-packed."""
    Mh, Kh = I // 2, H // 2
    W11, W12 = w[:Mh, :Kh], w[:Mh, Kh:]
    W21, W22 = w[Mh:, :Kh], w[Mh:, Kh:]
    combos = [W11 + W22, W21 + W22, W11, W22, W11 + W12, W21 - W11, W12 - W22]
    out = np.empty((7, MI2, P, KH2 * P), dtype=DT)
    for i, a in enumerate(combos):
        out[i] = a.reshape(MI2, P, KH2, P).transpose(0, 3, 2, 1).reshape(MI2, P, KH2 * P)
    return out


def _route(xf, gate_w, top_k):
    """Host router: returns per-expert (token_indices, weights)."""
    logits = xf @ gate_w.T.astype(np.float32)  # [T, E]
    m = logits.max(-1, keepdims=True)
    p = np.exp(logits - m)
    p /= p.sum(-1, keepdims=True)
    k = int(top_k)
    if k >= E:
        top_i = np.tile(np.arange(E), (xf.shape[0], 1))
    else:
        top_i = np.argpartition(-p, k, axis=-1)[:, :k]
    top_w = np.take_along_axis(p, top_i, axis=-1)
    top_w = top_w / top_w.sum(-1, keepdims=True)
    idxs, wts = [], []
    for e in range(E):
        sel = top_i == e  # [T, k]
        tok = np.nonzero(sel.any(-1))[0]
        w = (top_w * sel).sum(-1)[tok].astype(np.float32)
        idxs.append(tok)
        wts.append(w)
    return idxs, wts


def _pack_w1(w):  # [I, H] -> [MI, P, KH*P]; lhsT tile (m,k)[p,f] = w[128m+f, 128k+p]
    return np.ascontiguousarray(
        w.reshape(MI, P, KH, P).transpose(0, 3, 2, 1).reshape(MI, P, KH * P)
    )


def _pack_w3(w):  # [H, I] -> [KH, P, MI*P]; lhsT tile (m2,k2)[p,f] = w[128m2+f, 128k2+p]
    return np.ascontiguousarray(
        w.reshape(KH, P, MI, P).transpose(0, 3, 2, 1).reshape(KH, P, MI * P)
    )


def kernel(x, gate_w, w_gate, w_up, w_down, top_k):
    from concourse.bass_utils import run_bass_kernel_spmd

    x = np.asarray(x, dtype=np.float32)
    gate_w = np.asarray(gate_w, dtype=np.float32)
    w_gate = np.asarray(w_gate, dtype=np.float32)
    w_up = np.asarray(w_up, dtype=np.float32)
    w_down = np.asarray(w_down, dtype=np.float32)
    shape = x.shape
    xf = x.reshape(-1, shape[-1])
    T = xf.shape[0]

    idxs, wts = _route(xf, gate_w, top_k)
    C = max(max(len(ix) for ix in idxs), NB)
    C = ((C + 7) // 8) * 8

    # Strassen mm1/mm2 (12.5% fewer FLOPs) measured 1924us vs 1907us for the
    # dense pipeline: instruction-count tax + phase-boundary DMA latency eat
    # the savings. Keep dense as default; BASS_MOE_STRASSEN opts in.
    use_strassen = C >= 2 * BLK and bool(os.environ.get("BASS_MOE_STRASSEN"))
    if use_strassen:
        nc = build_program_strassen(C)
    else:
        nc = build_program(C)

    xf_bf = xf.astype(DT)
    in_maps = []
    for e in range(E):
        tok = idxs[e]
        xg = np.zeros((C, H), dtype=DT)
        xg[: len(tok)] = xf_bf[tok]
        # [C, H] -> x[p, k, t] = xg[t, 128k+p]
        xp = np.ascontiguousarray(xg.reshape(C, KH, P).transpose(2, 1, 0))
        if use_strassen:
            in_maps.append(
                {
                    "x": xp,
                    "ag": _pack_strassen_w1(w_gate[e]),
                    "au": _pack_strassen_w1(w_up[e]),
                    "wd": _pack_w3(w_down[e].astype(DT)),
                }
            )
        else:
            in_maps.append(
                {
                    "x": xp,
                    "wg": _pack_w1(w_gate[e].astype(DT)),
                    "wu": _pack_w1(w_up[e].astype(DT)),
                    "wd": _pack_w3(w_down[e].astype(DT)),
                }
            )

    trace = bool(os.environ.get("BASS_TRACE"))
    if trace:
        try:
            import antenv.axon_hooks  # noqa: F401  (trace path needs it under axon)
        except ImportError:
            # image lacks antenv.axon_hooks; build the NTFF hook directly
            # from the injected libaxon_pjrt.so and register a stub module
            try:
                import sys
                import types

                from trn_agent_boot.trn_boot import _ntff_profile_via_ctypes

                _hook = _ntff_profile_via_ctypes("/opt/axon/libaxon_pjrt.so")
                if _hook is None:
                    raise ImportError("no axon_start_nrt_profile symbol")
                _m = types.ModuleType("antenv.axon_hooks")
                _m.get_axon_ntff_profile_hook = lambda: _hook
                _m.set_axon_ntff_profile_hook = lambda h: None
                import antenv as _antenv

                sys.modules["antenv.axon_hooks"] = _m
                _antenv.axon_hooks = _m
            except Exception:
                trace = False
                os.environ["BASS_NEVER_TRACE"] = "1"
    res = run_bass_kernel_spmd(nc, in_maps, list(range(E)), trace=trace)
    globals()["LAST_RESULT"] = res

    out = np.zeros((T, H), dtype=np.float32)
    for e in range(E):
        tok = idxs[e]
        y = res.results[e]["y"]  # [P, KH, C]
        yt = y.transpose(2, 1, 0).reshape(C, H)[: len(tok)]
        out[tok] += yt * wts[e][:, None]
    return out.reshape(shape)



# revision 29
# speedup vs baseline: 1.0018x; 1.0018x over previous
"""MoE layer (8 experts, top-2) on 8 Trainium2 NeuronCores, expert-parallel.

Strategy
--------
Host (dispatch): compute router logits/top-k on host, gather each expert's
tokens into a padded capacity buffer (C = max expert load, 64-aligned),
pre-pack activations/weights into the exact SBUF tile layout
(partition-major) in fp16.
Device (one expert per core, SPMD): Y_e = w_down[e] @ (silu(w_gate[e] @ x_e)
* (w_up[e] @ x_e)) over the expert's C gathered tokens; all matmuls fp16
inputs with fp32 PSUM accumulation (fp16 runs at full PE rate like bf16 but
with 8x finer mantissa). Token columns are processed in 512-wide blocks, the
remainder merged into the last block so both share one pass over the
restreamed weights.
Host (combine): scatter-add per-token routing-weighted outputs.
"""

import os
import numpy as np
from contextlib import ExitStack

H = 2048
I = 5632
E = 8
P = 128
NB = 512  # token block (matmul free dim / PSUM bank)

KH = H // P   # 16  k-tiles over H
MI = I // P   # 44  m-tiles over I

DT = np.float16  # fp16: PE full rate like bf16, 8x finer mantissa


def _superblocks(C):
    """Column groups; a trailing remainder (<NB) is merged into the last
    full block so both share one pass over the weights."""
    blocks = []
    t = 0
    while t < C:
        blocks.append((t, min(NB, C - t)))
        t += NB
    sbs = [[b] for b in blocks]
    if len(sbs) >= 2 and sbs[-1][0][1] < NB:
        tail = sbs.pop()[0]
        sbs[-1].append(tail)
    return sbs


def build_program(C, h=H, i_dim=I, sim_safe_act=False):
    """Build the SPMD bass program for one expert over C tokens.

    DRAM I/O layouts (all partition-major, pre-packed on host):
      x  [P, KH, C]        fp16   x[p, k, t]  = token t, hidden 128k+p
      wg [MI, P, KH*P]     fp16   wg[m, p, kf] (kf = k*128+f): w_gate.T tiles
      wu [MI, P, KH*P]     fp16   same for w_up
      wd [KH, P, MI*P]     fp16   w_down.T tiles
      y  [P, KH, C]        f32    y[p, m2, t] = output hidden 128*m2+p
    """
    from concourse import bacc, tile, mybir

    kh = h // P
    mi = i_dim // P
    bf = mybir.dt.float16
    f32 = mybir.dt.float32
    Silu = mybir.ActivationFunctionType.Silu

    nc = bacc.Bacc(None)
    X = nc.declare_dram_parameter("x", [P, kh, C], bf, isOutput=False)
    WG = nc.declare_dram_parameter("wg", [mi, P, kh * P], bf, isOutput=False)
    WU = nc.declare_dram_parameter("wu", [mi, P, kh * P], bf, isOutput=False)
    WD = nc.declare_dram_parameter("wd", [kh, P, mi * P], bf, isOutput=False)
    Y = nc.declare_dram_parameter("y", [P, kh, C], f32, isOutput=True)

    with ExitStack() as ctx:
        tc = ctx.enter_context(tile.TileContext(nc))
        xpool = ctx.enter_context(tc.tile_pool(name="xpool", bufs=2))
        wpool = ctx.enter_context(tc.tile_pool(name="wpool", bufs=6))
        dpool = ctx.enter_context(tc.tile_pool(name="dpool", bufs=4))
        hpool = ctx.enter_context(tc.tile_pool(name="hpool", bufs=1))
        apool = ctx.enter_context(tc.tile_pool(name="apool", bufs=3))
        ypool = ctx.enter_context(tc.tile_pool(name="ypool", bufs=3))
        pg_pool = ctx.enter_context(tc.tile_pool(name="pg", bufs=3, space="PSUM"))
        pu_pool = ctx.enter_context(tc.tile_pool(name="pu", bufs=3, space="PSUM"))
        py_pool = ctx.enter_context(tc.tile_pool(name="py", bufs=2, space="PSUM"))
        wupool = ctx.enter_context(tc.tile_pool(name="wupool", bufs=1))

        # warmup: dummy matmul chain on garbage SBUF ramps the PE clock
        # p-state while the first weight/x DMAs are still in flight
        wu_w = wupool.tile([P, P], bf, tag="wu_w", name="wu_w")
        wu_x = wupool.tile([P, NB], bf, tag="wu_x", name="wu_x")
        nc.vector.memset(wu_w[:, :], 0.0)
        nc.vector.memset(wu_x[:, :], 0.0)
        wu_p = pg_pool.tile([P, NB], f32, tag="pg")
        for k in range(kh):
            nc.tensor.matmul(wu_p[:, :], wu_w[:, :], wu_x[:, :],
                             start=(k == 0), stop=(k == kh - 1))

        first_sb = True
        wg0 = wu0 = None
        for sb in _superblocks(C):
            q = kh * P // 4
            if first_sb:
                # prefetch m=0 weights ahead of the x stream so the PE's
                # first chain starts ~1us after launch, not behind 2MB of x
                wg0 = wpool.tile([P, kh * P], bf, tag="wg_t")
                for j in range(4):
                    nc.sync.dma_start(wg0[:, j * q : (j + 1) * q], WG[0, :, j * q : (j + 1) * q])
                wu0 = wpool.tile([P, kh * P], bf, tag="wu_t")
                for j in range(4):
                    nc.sync.dma_start(wu0[:, j * q : (j + 1) * q], WU[0, :, j * q : (j + 1) * q])
            # ---- load X for each column group: kh tiles [P, tn]
            x_ts = []
            for g, (t0, tn) in enumerate(sb):
                x_t = xpool.tile([P, kh, tn], bf, tag=f"x_t{g}", name=f"x_t{g}")
                for k in range(kh):
                    eng = nc.scalar if k % 2 == 0 else nc.gpsimd
                    eng.dma_start(x_t[:, k, :tn], X[:, k, t0 : t0 + tn])
                x_ts.append(x_t)

            # ---- mm1/mm2 + silu*mul -> h (one weight pass for all groups)
            h_ts = [
                hpool.tile([P, mi, sb[g][1]], bf, tag=f"h{g}", name=f"h_t{g}")
                for g in range(len(sb))
            ]
            for m in range(mi):
                if first_sb and m == 0:
                    wg_t, wu_t = wg0, wu0
                else:
                    wg_t = wpool.tile([P, kh * P], bf, tag="wg_t")
                    for j in range(4):
                        nc.sync.dma_start(wg_t[:, j * q : (j + 1) * q], WG[m, :, j * q : (j + 1) * q])
                    wu_t = wpool.tile([P, kh * P], bf, tag="wu_t")
                    for j in range(4):
                        nc.sync.dma_start(wu_t[:, j * q : (j + 1) * q], WU[m, :, j * q : (j + 1) * q])

                pgs, pus = [], []
                for g, (t0, tn) in enumerate(sb):
                    pg = pg_pool.tile([P, NB], f32, tag="pg")
                    pgs.append(pg)
                    for k in range(kh):
                        nc.tensor.matmul(
                            pg[:, :tn],
                            wg_t[:, k * P : (k + 1) * P],
                            x_ts[g][:, k, :tn],
                            start=(k == 0),
                            stop=(k == kh - 1),
                        )
                for g, (t0, tn) in enumerate(sb):
                    pu = pu_pool.tile([P, NB], f32, tag="pu")
                    pus.append(pu)
                    for k in range(kh):
                        nc.tensor.matmul(
                            pu[:, :tn],
                            wu_t[:, k * P : (k + 1) * P],
                            x_ts[g][:, k, :tn],
                            start=(k == 0),
                            stop=(k == kh - 1),
                        )
                for g, (t0, tn) in enumerate(sb):
                    pg, pu = pgs[g], pus[g]
                    g_act = apool.tile([P, NB], f32, tag="g_act")
                    if sim_safe_act:
                        # silu(g) = g * sigmoid(g); CoreSim lacks the Silu LUT
                        nc.scalar.activation(
                            g_act[:, :tn],
                            pg[:, :tn],
                            mybir.ActivationFunctionType.Sigmoid,
                        )
                        nc.vector.tensor_mul(g_act[:, :tn], g_act[:, :tn], pg[:, :tn])
                    else:
                        nc.scalar.activation(g_act[:, :tn], pg[:, :tn], Silu)
                    nc.vector.tensor_mul(h_ts[g][:, m, :tn], g_act[:, :tn], pu[:, :tn])

            # ---- mm3 -> y (one weight pass for all groups)
            for m2 in range(kh):
                dhalf = mi * P // 2
                wd_t = dpool.tile([P, mi * P], bf, tag="wd_t")
                nc.sync.dma_start(wd_t[:, :dhalf], WD[m2, :, :dhalf])
                nc.sync.dma_start(wd_t[:, dhalf:], WD[m2, :, dhalf:])
                # tail group first: its py chains are slot-constrained, so
                # bury them behind the full-rate main-group stream
                for g, (t0, tn) in reversed(list(enumerate(sb))):
                    py = py_pool.tile([P, NB], f32, tag="py")
                    for k2 in range(mi):
                        nc.tensor.matmul(
                            py[:, :tn],
                            wd_t[:, k2 * P : (k2 + 1) * P],
                            h_ts[g][:, k2, :tn],
                            start=(k2 == 0),
                            stop=(k2 == mi - 1),
                        )
                    y_sb = ypool.tile([P, NB], f32, tag="y_sb")
                    nc.vector.tensor_copy(y_sb[:, :tn], py[:, :tn])
                    nc.scalar.dma_start(Y[:, m2, t0 : t0 + tn], y_sb[:, :tn])
            first_sb = False

    nc.compile()
    return nc


MI2 = MI // 2  # 22 m-tiles over I/2
KH2 = KH // 2  # 8 k-tiles over H/2
BLK = 960      # strassen column block (halves of 480 fit one PSUM bank; a
               # C=2104 tail of 184 has 92-wide halves, wide enough that
               # LDWEIGHTS still hides under the tail matmuls)


def build_program_strassen(C):
    """Strassen(1-level) mm1/mm2 + dense mm3, h staged through DRAM.

    Columns: two BLK blocks + tail (C - 2*BLK, halves tail//2), tail merged
    into pass 1 so it shares that pass's A-weight stream.

    DRAM I/O (fp16 unless noted):
      x  [P, KH, C]            x[p,k,t] = token t, hidden 128k+p
      ag [7, MI2, P, KH2*P]    Strassen A-combos of w_gate (lhsT tiles)
      au [7, MI2, P, KH2*P]    same for w_up
      wd [KH, P, MI*P]         w_down.T tiles (dense mm3)
      y  [P, KH, C]  f32       output
      hd [P, MI, C]            internal scratch: h = silu(g)*u
    """
    from concourse import bacc, tile, mybir

    bf = mybir.dt.float16
    f32 = mybir.dt.float32
    Silu = mybir.ActivationFunctionType.Silu

    assert C >= 2 * BLK
    tail = C - 2 * BLK
    th = tail // 2
    hn = BLK // 2

    nc = bacc.Bacc(None)
    X = nc.declare_dram_parameter("x", [P, KH, C], bf, isOutput=False)
    AG = nc.declare_dram_parameter("ag", [7, MI2, P, KH2 * P], bf, isOutput=False)
    AU = nc.declare_dram_parameter("au", [7, MI2, P, KH2 * P], bf, isOutput=False)
    WD = nc.declare_dram_parameter("wd", [KH, P, MI * P], bf, isOutput=False)
    Y = nc.declare_dram_parameter("y", [P, KH, C], f32, isOutput=True)
    HD = nc.dram_tensor("hd", [P, MI, C], bf)

    with ExitStack() as ctx:
        tc = ctx.enter_context(tile.TileContext(nc))
        xpool = ctx.enter_context(tc.tile_pool(name="xpool", bufs=1))
        xtpool = ctx.enter_context(tc.tile_pool(name="xtpool", bufs=1))
        spool = ctx.enter_context(tc.tile_pool(name="spool", bufs=1))
        stpool = ctx.enter_context(tc.tile_pool(name="stpool", bufs=1))
        apool = ctx.enter_context(tc.tile_pool(name="apool", bufs=8))
        gupool = ctx.enter_context(tc.tile_pool(name="gupool", bufs=2))
        gutpool = ctx.enter_context(tc.tile_pool(name="gutpool", bufs=2))
        hopool = ctx.enter_context(tc.tile_pool(name="hopool", bufs=2))
        hotpool = ctx.enter_context(tc.tile_pool(name="hotpool", bufs=2))
        hipool = ctx.enter_context(tc.tile_pool(name="hipool", bufs=3))
        htpool = ctx.enter_context(tc.tile_pool(name="htpool", bufs=1))
        dpool = ctx.enter_context(tc.tile_pool(name="dpool", bufs=2))
        ypool = ctx.enter_context(tc.tile_pool(name="ypool", bufs=2))
        wupool = ctx.enter_context(tc.tile_pool(name="wupool", bufs=1))
        ppool = ctx.enter_context(tc.tile_pool(name="pp", bufs=8, space="PSUM"))

        # warmup: dummy chain ramps PE clock while first DMAs fly
        wu_w = wupool.tile([P, P], bf, tag="wu_w", name="wu_w")
        wu_x = wupool.tile([P, NB], bf, tag="wu_x", name="wu_x")
        nc.vector.memset(wu_w[:, :], 0.0)
        nc.vector.memset(wu_x[:, :], 0.0)
        for _ in range(4):
            wu_p = ppool.tile([P, NB], f32, tag="pp")
            for k in range(KH):
                nc.tensor.matmul(wu_p[:, :], wu_w[:, :], wu_x[:, :],
                                 start=(k == 0), stop=(k == KH - 1))

        def emit_combines(eng, P_, oA, oB, w):
            """G-quadrant combines; DVE may read only ONE PSUM operand per op,
            so accumulate into SBUF. Op order frees PSUM banks ahead of the
            next product round's ring slots (P1 first, then P2, P7, P4, ...).
            oA = row-block m [P, 2w] (C11|C12), oB = row-block m+MI2 (C21|C22)."""
            A_lo, A_hi = oA[:, 0:w], oA[:, w:2 * w]
            B_lo, B_hi = oB[:, 0:w], oB[:, w:2 * w]
            p = [t[:, 0:w] for t in P_]
            eng.tensor_copy(A_lo, p[0])            # C11 := P1
            eng.tensor_copy(B_hi, p[0])            # C22 := P1        [P1 free]
            eng.tensor_sub(B_hi, B_hi, p[1])       # C22 -= P2
            eng.tensor_copy(B_lo, p[1])            # C21 := P2        [P2 free]
            eng.tensor_add(A_lo, A_lo, p[6])       # C11 += P7        [P7 free]
            eng.tensor_add(B_lo, B_lo, p[3])       # C21 += P4
            eng.tensor_add(A_lo, A_lo, p[3])       # C11 += P4        [P4 free]
            eng.tensor_add(B_hi, B_hi, p[2])       # C22 += P3
            eng.tensor_copy(A_hi, p[2])            # C12 := P3        [P3 free]
            eng.tensor_add(A_hi, A_hi, p[4])       # C12 += P5
            eng.tensor_sub(A_lo, A_lo, p[4])       # C11 -= P5        [P5 free]
            eng.tensor_add(B_hi, B_hi, p[5])       # C22 += P6        [P6 free]

        for pidx in range(2):
            t0 = pidx * BLK
            has_tail = (pidx == 1) and tail > 0

            # ---- x for this block (+ tail)
            x_t = xpool.tile([P, KH, BLK], bf, tag="x", name=f"x_b{pidx}")
            for k in range(KH):
                eng = nc.scalar if k % 2 == 0 else nc.gpsimd
                eng.dma_start(x_t[:, k, :], X[:, k, t0:t0 + BLK])
            if has_tail:
                xt = xtpool.tile([P, KH, tail], bf, tag="xt", name="x_tail")
                nc.gpsimd.dma_start(xt[:, :, :], X[:, :, 2 * BLK:2 * BLK + tail])

            # ---- S combos (vector); S2/S5 are raw views of x
            def build_s(xsrc, w, pool, tg):
                s1 = pool.tile([P, KH2, w], bf, tag=f"s1{tg}")
                nc.vector.tensor_add(s1[:, :, :], xsrc[:, 0:KH2, 0:w], xsrc[:, KH2:KH, w:2 * w])
                s3 = pool.tile([P, KH2, w], bf, tag=f"s3{tg}")
                nc.vector.tensor_sub(s3[:, :, :], xsrc[:, 0:KH2, w:2 * w], xsrc[:, KH2:KH, w:2 * w])
                s4 = pool.tile([P, KH2, w], bf, tag=f"s4{tg}")
                nc.vector.tensor_sub(s4[:, :, :], xsrc[:, KH2:KH, 0:w], xsrc[:, 0:KH2, 0:w])
                s6 = pool.tile([P, KH2, w], bf, tag=f"s6{tg}")
                nc.vector.tensor_add(s6[:, :, :], xsrc[:, 0:KH2, 0:w], xsrc[:, 0:KH2, w:2 * w])
                s7 = pool.tile([P, KH2, w], bf, tag=f"s7{tg}")
                nc.vector.tensor_add(s7[:, :, :], xsrc[:, KH2:KH, 0:w], xsrc[:, KH2:KH, w:2 * w])
                # mov(i, k) -> AP for product i, k-tile k
                def mov(i, k):
                    return [
                        lambda: s1[:, k, :],
                        lambda: xsrc[:, k, 0:w],            # S2 = X11
                        lambda: s3[:, k, :],
                        lambda: s4[:, k, :],
                        lambda: xsrc[:, KH2 + k, w:2 * w],  # S5 = X22
                        lambda: s6[:, k, :],
                        lambda: s7[:, k, :],
                    ][i]()
                return mov

            mov = build_s(x_t, hn, spool, "")
            if has_tail:
                movt = build_s(xt, th, stpool, "t")

            # ---- mm1/mm2 strassen m-loop
            for m in range(MI2):
                for W_, oAB, oABt in ((AG, "g", "gt"), (AU, "u", "ut")):
                    a_ts = []
                    for i in range(7):
                        a_t = apool.tile([P, KH2 * P], bf, tag="a")
                        half = KH2 * P // 2
                        nc.sync.dma_start(a_t[:, :half], W_[i, m, :, :half])
                        nc.gpsimd.dma_start(a_t[:, half:], W_[i, m, :, half:])
                        a_ts.append(a_t)
                    # block products
                    P_ = []
                    for i in range(7):
                        pp = ppool.tile([P, NB], f32, tag="pp")
                        for k in range(KH2):
                            nc.tensor.matmul(
                                pp[:, :hn], a_ts[i][:, k * P:(k + 1) * P], mov(i, k),
                                start=(k == 0), stop=(k == KH2 - 1))
                        P_.append(pp)
                    oA = gupool.tile([P, BLK], bf, tag=f"{oAB}A")
                    oB = gupool.tile([P, BLK], bf, tag=f"{oAB}B")
                    emit_combines(nc.vector, P_, oA, oB, hn)
                    if oAB == "g":
                        gA, gB = oA, oB
                    else:
                        uA, uB = oA, oB
                    if has_tail:
                        Pt = []
                        for i in range(7):
                            pp = ppool.tile([P, NB], f32, tag="pp")
                            for k in range(KH2):
                                nc.tensor.matmul(
                                    pp[:, :th], a_ts[i][:, k * P:(k + 1) * P], movt(i, k),
                                    start=(k == 0), stop=(k == KH2 - 1))
                            Pt.append(pp)
                        oAt = gutpool.tile([P, tail], bf, tag=f"{oABt}A")
                        oBt = gutpool.tile([P, tail], bf, tag=f"{oABt}B")
                        emit_combines(nc.vector, Pt, oAt, oBt, th)
                        if oAB == "g":
                            gAt, gBt = oAt, oBt
                        else:
                            uAt, uBt = oAt, oBt

                # silu in-place on g, then h = g*u -> DRAM
                nc.scalar.activation(gA[:, :], gA[:, :], Silu)
                nc.scalar.activation(gB[:, :], gB[:, :], Silu)
                hA = hopool.tile([P, BLK], bf, tag="hA")
                nc.gpsimd.tensor_mul(hA[:, :], gA[:, :], uA[:, :])
                nc.scalar.dma_start(HD[:, m, t0:t0 + BLK], hA[:, :])
                hB = hopool.tile([P, BLK], bf, tag="hB")
                nc.gpsimd.tensor_mul(hB[:, :], gB[:, :], uB[:, :])
                nc.scalar.dma_start(HD[:, m + MI2, t0:t0 + BLK], hB[:, :])
                if has_tail:
                    nc.scalar.activation(gAt[:, :], gAt[:, :], Silu)
                    nc.scalar.activation(gBt[:, :], gBt[:, :], Silu)
                    hAt = hotpool.tile([P, tail], bf, tag="hAt")
                    nc.vector.tensor_mul(hAt[:, :], gAt[:, :], uAt[:, :])
                    nc.scalar.dma_start(HD[:, m, 2 * BLK:2 * BLK + tail], hAt[:, :])
                    hBt = hotpool.tile([P, tail], bf, tag="hBt")
                    nc.vector.tensor_mul(hBt[:, :], gBt[:, :], uBt[:, :])
                    nc.scalar.dma_start(HD[:, m + MI2, 2 * BLK:2 * BLK + tail], hBt[:, :])

            # ---- mm3 (dense) for this block (+ tail in pass 1)
            chunks = [(0, 160), (160, 160), (320, 160), (480, 160), (640, 160), (800, 160)]
            groups = [chunks[0:3], chunks[3:6]]
            third = (MI // 3) + 1  # 15
            for gi, group in enumerate(groups):
                h_ins = []
                for (co, cn) in group:
                    hq = hipool.tile([P, MI, cn], bf, tag="hq")
                    # split the chunk load across all three DMA queues
                    nc.gpsimd.dma_start(hq[:, :third, :], HD[:, :third, t0 + co:t0 + co + cn])
                    nc.sync.dma_start(hq[:, third:2 * third, :], HD[:, third:2 * third, t0 + co:t0 + co + cn])
                    nc.scalar.dma_start(hq[:, 2 * third:, :], HD[:, 2 * third:, t0 + co:t0 + co + cn])
                    h_ins.append(hq)
                tail_here = has_tail and gi == len(groups) - 1
                if tail_here:
                    htq = htpool.tile([P, MI, tail], bf, tag="htq")
                    nc.gpsimd.dma_start(htq[:, :, :], HD[:, :, 2 * BLK:2 * BLK + tail])
                for m2 in range(KH):
                    dhalf = MI * P // 2
                    wd_t = dpool.tile([P, MI * P], bf, tag="wd_t")
                    nc.sync.dma_start(wd_t[:, :dhalf], WD[m2, :, :dhalf])
                    nc.gpsimd.dma_start(wd_t[:, dhalf:], WD[m2, :, dhalf:])
                    todo = [(co, cn, hq) for (co, cn), hq in zip(group, h_ins)]
                    if tail_here:
                        todo.append((2 * BLK - t0, tail, htq))
                    for (co, cn, hq) in todo:
                        py = ppool.tile([P, NB], f32, tag="pp")
                        for k2 in range(MI):
                            nc.tensor.matmul(
                                py[:, :cn], wd_t[:, k2 * P:(k2 + 1) * P], hq[:, k2, :cn],
                                start=(k2 == 0), stop=(k2 == MI - 1))
                        y_sb = ypool.tile([P, max(160, tail)], f32, tag="y_sb")
                        nc.vector.tensor_copy(y_sb[:, :cn], py[:, :cn])
                        nc.scalar.dma_start(Y[:, m2, t0 + co:t0 + co + cn], y_sb[:, :cn])

    nc.compile()
    return nc


def _pack_strassen_w1(w):
    """[I, H] -> [7, MI2, P, KH2*P] strassen A-combos, lhsT-packed."""
    Mh, Kh = I // 2, H // 2
    W11, W12 = w[:Mh, :Kh], w[:Mh, Kh:]
    W21, W22 = w[Mh:, :Kh], w[Mh:, Kh:]
    combos = [W11 + W22, W21 + W22, W11, W22, W11 + W12, W21 - W11, W12 - W22]
    out = np.empty((7, MI2, P, KH2 * P), dtype=DT)
    for i, a in enumerate(combos):
        out[i] = a.reshape(MI2, P, KH2, P).transpose(0, 3, 2, 1).reshape(MI2, P, KH2 * P)
    return out


def _route(xf, gate_w, top_k):
    """Host router: returns per-expert (token_indices, weights)."""
    logits = xf @ gate_w.T.astype(np.float32)  # [T, E]
    m = logits.max(-1, keepdims=True)
    p = np.exp(logits - m)
    p /= p.sum(-1, keepdims=True)
    k = int(top_k)
    if k >= E:
        top_i = np.tile(np.arange(E), (xf.shape[0], 1))
    else:
        top_i = np.argpartition(-p, k, axis=-1)[:, :k]
    top_w = np.take_along_axis(p, top_i, axis=-1)
    top_w = top_w / top_w.sum(-1, keepdims=True)
    idxs, wts = [], []
    for e in range(E):
        sel = top_i == e  # [T, k]
        tok = np.nonzero(sel.any(-1))[0]
        w = (top_w * sel).sum(-1)[tok].astype(np.float32)
        idxs.append(tok)
        wts.append(w)
    return idxs, wts


def _pack_w1(w):  # [I, H] -> [MI, P, KH*P]; lhsT tile (m,k)[p,f] = w[128m+f, 128k+p]
    return np.ascontiguousarray(
        w.reshape(MI, P, KH, P).transpose(0, 3, 2, 1).reshape(MI, P, KH * P)
    )


def _pack_w3(w):  # [H, I] -> [KH, P, MI*P]; lhsT tile (m2,k2)[p,f] = w[128m2+f, 128k2+p]
    return np.ascontiguousarray(
        w.reshape(KH, P, MI, P).transpose(0, 3, 2, 1).reshape(KH, P, MI * P)
    )


def kernel(x, gate_w, w_gate, w_up, w_down, top_k):
    from concourse.bass_utils import run_bass_kernel_spmd

    x = np.asarray(x, dtype=np.float32)
    gate_w = np.asarray(gate_w, dtype=np.float32)
    w_gate = np.asarray(w_gate, dtype=np.float32)
    w_up = np.asarray(w_up, dtype=np.float32)
    w_down = np.asarray(w_down, dtype=np.float32)
    shape = x.shape
    xf = x.reshape(-1, shape[-1])
    T = xf.shape[0]

    idxs, wts = _route(xf, gate_w, top_k)
    C = max(max(len(ix) for ix in idxs), NB)
    C = ((C + 7) // 8) * 8

    # Strassen mm1/mm2 (12.5% fewer FLOPs) measured 1924us vs 1907us for the
    # dense pipeline: instruction-count tax + phase-boundary DMA latency eat
    # the savings. Keep dense as default; BASS_MOE_STRASSEN opts in.
    use_strassen = C >= 2 * BLK and bool(os.environ.get("BASS_MOE_STRASSEN"))
    if use_strassen:
        nc = build_program_strassen(C)
    else:
        nc = build_program(C)

    xf_bf = xf.astype(DT)
    in_maps = []
    for e in range(E):
        tok = idxs[e]
        xg = np.zeros((C, H), dtype=DT)
        xg[: len(tok)] = xf_bf[tok]
        # [C, H] -> x[p, k, t] = xg[t, 128k+p]
        xp = np.ascontiguousarray(xg.reshape(C, KH, P).transpose(2, 1, 0))
        if use_strassen:
            in_maps.append(
                {
                    "x": xp,
                    "ag": _pack_strassen_w1(w_gate[e]),
                    "au": _pack_strassen_w1(w_up[e]),
                    "wd": _pack_w3(w_down[e].astype(DT)),
                }
            )
        else:
            in_maps.append(
                {
                    "x": xp,
                    "wg": _pack_w1(w_gate[e].astype(DT)),
                    "wu": _pack_w1(w_up[e].astype(DT)),
                    "wd": _pack_w3(w_down[e].astype(DT)),
                }
            )

    trace = bool(os.environ.get("BASS_TRACE"))
    if trace:
        try:
            import antenv.axon_hooks  # noqa: F401  (trace path needs it under axon)
        except ImportError:
            # image lacks antenv.axon_hooks; build the NTFF hook directly
            # from the injected libaxon_pjrt.so and register a stub module
            try:
                import sys
                import types

                from trn_agent_boot.trn_boot import _ntff_profile_via_ctypes

                _hook = _ntff_profile_via_ctypes("/opt/axon/libaxon_pjrt.so")
                if _hook is None:
                    raise ImportError("no axon_start_nrt_profile symbol")
                _m = types.ModuleType("antenv.axon_hooks")
                _m.get_axon_ntff_profile_hook = lambda: _hook
                _m.set_axon_ntff_profile_hook = lambda h: None
                import antenv as _antenv

                sys.modules["antenv.axon_hooks"] = _m
                _antenv.axon_hooks = _m
            except Exception:
                trace = False
                os.environ["BASS_NEVER_TRACE"] = "1"
    res = run_bass_kernel_spmd(nc, in_maps, list(range(E)), trace=trace)
    globals()["LAST_RESULT"] = res

    out = np.zeros((T, H), dtype=np.float32)
    for e in range(E):
        tok = idxs[e]
        y = res.results[e]["y"]  # [P, KH, C]
        yt = y.transpose(2, 1, 0).reshape(C, H)[: len(tok)]
        out[tok] += yt * wts[e][:, None]
    return out.reshape(shape)

